# revision 83
# baseline (speedup 1.0000x reference)
"""Trainium2 Bass kernel for nn_CrossAttentionFusion.

Math. With a single-token key/value axis, softmax over that axis is exactly
1.0, so each cross-attention path collapses to its V/out projections:

    z_i = x_kv @ W_i^T + c_i,  W_i = w_o_i @ wv_i.

The LayerNorm + projection head folds completely through host-side algebra:

    h = rs * (Wg @ z - mu * wbar) + b2,    Wg = w_proj * g
      = rs * (A1 @ xu + A2 @ xm + chat) + b2
  with A_i = Wg_i @ W_i - outer(wbar, s_i)/2d  (the -mu*wbar rank-1 term is
  folded into A via mu = (s1.xu + s2.xm + sumc)/2d), leaving only the row
  variance as extra device work:

    var = (xu' G1 xu + xm' G2 xm + lv.x + const)/2d - mu^2,   G_i = W_i^T W_i.

  The quadratic forms use the eigen square root P_i = sqrt(L) U^T of G_i,
  truncated to rank R (tail mean folded into the constant): u = P x, then
  v = sum(u^2) via the Act engine's Square+accum. All matmuls run as fp8
  e4m3 DoubleRow (2 k-tiles per instruction); the main matmuls use a 3-term
  hi/lo split (Ah@(xh+xl) + Al@xh) which matches bf16 accuracy.

Device layout is batch-major (out[batch_p, feat_f]): LN scalars are
per-partition scalars, so the tail is a single Act gelu(scale=rs) per PSUM
half and there are no on-chip broadcasts or transposes.
"""

import sys

sys.path.insert(0, "/opt/trn_rl_repo")

import ml_dtypes
import numpy as np

import concourse.bass as bass
import concourse.mybir as mybir
import concourse.tile as tile
from concourse.bass_utils import run_bass_kernel_spmd

N_CORES = 8
B = 16384
D = 1024
BC = B // N_CORES          # batch rows per core (2048)
NCHUNK = 512               # batch rows per x-DMA chunk
NCH = BC // NCHUNK         # chunks per core (4)
NBT = NCHUNK // 128        # b-tiles per chunk (4)
KT = D // 128              # k tiles (8)
KP = KT // 2               # k pairs for DoubleRow (4)
R = 384                    # variance quadratic-form rank
RW = R + 2                 # quad rhs width per path: P rows + [s, lv] columns
RXL = 256                  # rank of the truncated-SVD xl term (chunks 1+)
GU = 8.0                   # fp8 scale between the two xl-term stages
LN_EPS = 1e-5
ALPHA = 128.0              # fp8 scale on A-hat
GAMMAP = 32.0              # fp8 scale on P

F32 = mybir.dt.float32
BF16 = mybir.dt.bfloat16
FP8 = mybir.dt.float8e4
nf8 = ml_dtypes.float8_e4m3
DR = mybir.MatmulPerfMode.DoubleRow


def split_multi_waits(nc):
    """This walrus build only honors one sync-wait per instruction. Move any
    extra waits onto same-engine NOPs inserted immediately before."""
    for f in nc.m.functions:
        for bb in f.blocks:
            new_insts = []
            changed = False
            for inst in bb.instructions:
                si = inst.sync_info
                waits = list(si.on_wait) if si and si.on_wait else []
                if len(waits) > 1:
                    changed = True
                    for w in waits[:-1]:
                        nop = mybir.InstNoOp(
                            name=nc.get_next_instruction_name(), ins=[], outs=[]
                        )
                        nop.engine = inst.engine
                        nop.sync_info = mybir.SyncInfo(on_wait=[w], on_update=[])
                        nc.register_instruction(nop)
                        new_insts.append(nop)
                    si.on_wait = waits[-1:]
                new_insts.append(inst)
            if changed:
                bb.instructions[:] = new_insts


def build_program(
    consts, skip_w_dma=False, skip_x_dma=False, skip_out_dma=False, warmup=0
):
    """consts: dict with mc, vc, epsp, sqrt_kv floats.
    skip_* flags are for TimelineSim component analysis only (perf.py)."""
    nc = bass.Bass("TRN2", target_bir_lowering=False, debug=False)

    # hi/lo packs: [NCH, D, 2, NCHUNK] with path (u, m) on axis 2, so a
    # chunk slice is a 3-dim DMA
    xh = nc.dram_tensor("xh", [NCH, D, 2, NCHUNK], FP8, kind="ExternalInput").ap()
    xl = nc.dram_tensor("xl", [NCH, D, 2, NCHUNK], FP8, kind="ExternalInput").ap()
    ah1 = nc.dram_tensor("ah1", [D, D], FP8, kind="ExternalInput").ap()
    al1 = nc.dram_tensor("al1", [D, D], FP8, kind="ExternalInput").ap()
    ah2 = nc.dram_tensor("ah2", [D, D], FP8, kind="ExternalInput").ap()
    al2 = nc.dram_tensor("al2", [D, D], FP8, kind="ExternalInput").ap()
    pp = nc.dram_tensor("pp", [D, 2 * RW], FP8, kind="ExternalInput").ap()
    vv = nc.dram_tensor("vv", [D, 2 * RXL], FP8, kind="ExternalInput").ap()
    ws1 = nc.dram_tensor("ws1", [RXL, D], FP8, kind="ExternalInput").ap()
    ws2 = nc.dram_tensor("ws2", [RXL, D], FP8, kind="ExternalInput").ap()
    out = nc.dram_tensor("out", [BC, D], F32, kind="ExternalOutput").ap()

    def kp3(t):
        return t.rearrange("(k p) n -> p k n", p=128)

    ALU = mybir.AluOpType
    AF = mybir.ActivationFunctionType

    with tile.TileContext(nc) as tc:
        with (
            tc.tile_pool(name="wconst", bufs=1) as wconst,
            tc.tile_pool(name="xin", bufs=2) as xin,
            tc.tile_pool(name="scr", bufs=2) as scrp,
            tc.tile_pool(name="stat", bufs=2) as statp,
            tc.tile_pool(name="outp", bufs=4) as outp,
            tc.tile_pool(name="yps", bufs=4, space="PSUM") as yps,
            tc.tile_pool(name="ups", bufs=3, space="PSUM") as ups,
            tc.tile_pool(name="uxps", bufs=1, space="PSUM") as uxps,
        ):
            # --- resident weights ---
            pp_sb = wconst.tile([128, KT, 2 * RW], FP8)
            vv_sb = wconst.tile([128, KT, 2 * RXL], FP8)
            ws1_sb = wconst.tile([128, 2, D], FP8)
            ws2_sb = wconst.tile([128, 2, D], FP8)
            ah1_sb = wconst.tile([128, KT, D], FP8)
            al1_sb = wconst.tile([128, KT, D], FP8)
            ah2_sb = wconst.tile([128, KT, D], FP8)
            al2_sb = wconst.tile([128, KT, D], FP8)
            eps_sb = wconst.tile([128, 1], F32)
            nc.vector.memset(eps_sb[:], consts["epsp"])
            # --- PE warmup: keep the tensor engine continuously busy during
            # the DMA head so the p-state clock is fully ramped when real
            # work arrives (matmul cost is 2-3.7x until 3us of busy).
            if warmup > 0:
                wu_sb = wconst.tile([128, 512], FP8)
                nc.vector.memset(wu_sb[:], 1.0)
                wups = ups.tile([128, 512], F32, tag="u")
                for i in range(warmup):
                    nc.tensor.matmul(
                        wups[:],
                        lhsT=wu_sb[:, 0:128],
                        rhs=wu_sb[:],
                        start=(i == 0), stop=(i == warmup - 1),
                        skip_group_check=True,
                    )

            # DMA order: mu/quad inputs first, then A-hi, xlo, A-lo.
            x_sb = {}  # (name, chunk) -> tile; xh/xl tiles pack (u, m) paths

            def make_x_tiles(ci):
                for name in ("xh", "xl"):
                    x_sb[(name, ci)] = xin.tile(
                        [128, KT, 2, NCHUNK], FP8, tag=name, name=f"{name}_{ci}"
                    )

            def load_x(ci, names, paths=(0, 1)):
                src = {"xh": xh, "xl": xl}
                for name in names:
                    t = x_sb[(name, ci)]
                    if not skip_x_dma:
                        s4 = src[name].rearrange(
                            "c (k p) two n -> c p k two n", p=128
                        )[ci]
                        for pi in paths:
                            nc.sync.dma_start(
                                t[:, :, pi, :], s4[:, :, pi, :]
                            )
                    else:
                        nc.vector.memset(t[:, 0, 0, 0:1], 0.0)

            def load_w(wt, wsb, jpair):
                ks = slice(4 * jpair, 4 * jpair + 4)
                nc.sync.dma_start(wsb[:, ks, :], kp3(wt)[:, ks, :])

            def load_w_half(wt, wsb, half):
                fs = slice(512 * half, 512 * (half + 1))
                nc.sync.dma_start(wsb[:, :, fs], kp3(wt)[:, :, fs])

            # head DMA order: quad inputs (k-split for earliest start),
            # A-hi, xl0 (per path), A-lo by half, then the xl-SVD weights.
            make_x_tiles(0)
            xh0 = x_sb[("xh", 0)]
            if not skip_x_dma:
                s4 = xh.rearrange("c (k p) two n -> c p k two n", p=128)[0]
                for kk in (slice(0, 4), slice(4, KT)):
                    nc.sync.dma_start(xh0[:, kk, 0, :], s4[:, kk, 0, :])
                    if not skip_w_dma:
                        nc.sync.dma_start(
                            pp_sb[:, kk, 0:RW], kp3(pp)[:, kk, 0:RW]
                        )
            else:
                nc.vector.memset(xh0[:, 0, 0, 0:1], 0.0)
            load_x(0, ("xh",), paths=(1,))
            if not skip_w_dma:
                nc.sync.dma_start(
                    pp_sb[:, :, RW : 2 * RW], kp3(pp)[:, :, RW : 2 * RW]
                )
            else:
                for t in (pp_sb, vv_sb, ws1_sb, ws2_sb,
                          ah1_sb, al1_sb, ah2_sb, al2_sb):
                    nc.vector.memset(t[:, 0, 0:1], 0.0)
            if not skip_w_dma:
                # A-hi first (terms 1-2 of mains), 2 k-pairs per DMA,
                # matrix-major to match the mains term consumption order
                for wt, wsb in ((ah1, ah1_sb), (ah2, ah2_sb)):
                    for jp in range(2):
                        load_w(wt, wsb, jp)
            load_x(0, ("xl",), paths=(0,))
            load_x(0, ("xl",), paths=(1,))
            if not skip_w_dma:
                # A-lo by half so the h0 passes never wait on h1 bytes
                for half in range(2):
                    for wt, wsb in ((al1, al1_sb), (al2, al2_sb)):
                        load_w_half(wt, wsb, half)
                # truncated-SVD xl weights (first needed by chunk 1)
                nc.sync.dma_start(vv_sb[:], kp3(vv)[:])
                nc.sync.dma_start(
                    ws1_sb[:], ws1.rearrange("(k p) n -> p k n", p=128)[:]
                )
                nc.sync.dma_start(
                    ws2_sb[:], ws2.rearrange("(k p) n -> p k n", p=128)[:]
                )

            # main terms in DMA arrival order: (weight tile, x tile, path idx)
            def mains_terms(xs):
                return (
                    (ah1_sb, xs["xh"], 0), (ah2_sb, xs["xh"], 1),
                    (ah1_sb, xs["xl"], 0), (ah2_sb, xs["xl"], 1),
                    (al1_sb, xs["xh"], 0), (al2_sb, xs["xh"], 1),
                )

            def emit_quad(xs, b, pi, vtag):
                """P@xh (with s/lv appended as 2 extra rhs columns), square+
                accum on Act over [0:R], mu columns copied out on DVE."""
                bt = bass.ts(b, 128)
                xh_t = xs["xh"]
                up = ups.tile([128, RW], F32, tag="u")
                for j in range(KP):
                    nc.tensor.matmul(
                        up[:],
                        lhsT=xh_t[:, 2 * j : 2 * j + 2, pi, bt],
                        rhs=pp_sb[:, 2 * j : 2 * j + 2, pi * RW : pi * RW + RW],
                        start=(j == 0), stop=(j == KP - 1), perf_mode=DR,
                    )
                sq = scrp.tile([128, R], BF16, tag="sq")
                va = statp.tile([128, 1], F32, tag=vtag, name=f"va_{vtag}")
                nc.scalar.activation(
                    sq[:], up[:, 0:R], AF.Square,
                    bias=0.0, scale=consts["sqrt_kv"], accum_out=va[:],
                )
                muv = statp.tile([128, 2], F32, tag=f"muv{vtag}",
                                 name=f"muv_{vtag}")
                nc.vector.tensor_copy(out=muv[:], in_=up[:, R : R + 2])
                return va, muv

            def emit_combine(vaccs, muvs, b):
                m_sb = statp.tile([128, 1], F32, tag="m")
                nc.vector.scalar_tensor_tensor(
                    out=m_sb[:], in0=muvs[0][:, 0:1], scalar=consts["mc"],
                    in1=muvs[1][:, 0:1], op0=ALU.add, op1=ALU.add,
                )
                msq = statp.tile([128, 1], F32, tag="msq")
                nc.vector.tensor_mul(msq[:], m_sb[:], m_sb[:])
                v1 = statp.tile([128, 1], F32, tag="v1")
                nc.vector.scalar_tensor_tensor(
                    out=v1[:], in0=muvs[0][:, 1:2], scalar=consts["vc"],
                    in1=muvs[1][:, 1:2], op0=ALU.add, op1=ALU.add,
                )
                v2 = statp.tile([128, 1], F32, tag="v2")
                nc.vector.scalar_tensor_tensor(
                    out=v2[:], in0=msq[:], scalar=-1.0,
                    in1=vaccs[0][:], op0=ALU.mult, op1=ALU.add,
                )
                v3 = statp.tile([128, 1], F32, tag="v3")
                nc.vector.tensor_tensor(
                    out=v3[:], in0=v1[:], in1=v2[:], op=ALU.add
                )
                varp = statp.tile([128, 1], F32, tag="varp")
                nc.vector.tensor_tensor(
                    out=varp[:], in0=v3[:], in1=vaccs[1][:], op=ALU.add
                )
                sd = statp.tile([128, 1], F32, tag="sd")
                nc.scalar.activation(sd[:], varp[:], AF.Sqrt, bias=eps_sb[:])
                rs_sb = statp.tile([128, 1], F32, tag=f"rs{b}")
                nc.vector.reciprocal(rs_sb[:], sd[:])
                return rs_sb

            def emit_xl_stage1(xs, b):
                """uT = [S^.5 V^T] @ xl per path, feature-major [RXL, 128b]
                in one psum bank (4 groups), fp8-copied to SBUF as the
                stage-2 lhsT."""
                bt = bass.ts(b, 128)
                xl_t = xs["xl"]
                ux = uxps.tile([128, 2, 2, 128], F32, tag="ux")
                for gi, (pi, rt) in enumerate(
                    ((0, 0), (0, 1), (1, 0), (1, 1))
                ):
                    c0 = pi * RXL + rt * 128
                    for j in range(KP):
                        nc.tensor.matmul(
                            ux[:, pi, rt, :],
                            lhsT=vv_sb[:, 2 * j : 2 * j + 2, c0 : c0 + 128],
                            rhs=xl_t[:, 2 * j : 2 * j + 2, pi, bt],
                            start=(gi == 0 and j == 0),
                            stop=(gi == 3 and j == KP - 1),
                            perf_mode=DR,
                            skip_group_check=True,
                        )
                uxs = scrp.tile([128, 2, 2, 128], FP8, tag="uxs",
                                name=f"uxs_{b}", bufs=NBT)
                nc.vector.tensor_copy(out=uxs[:], in_=ux[:])
                return uxs

            def emit_mains_terms(xs, b, yp, half, tis, start, stop):
                bt = bass.ts(b, 128)
                fs = slice(512 * half, 512 * (half + 1))
                terms = mains_terms(xs)
                tl = [terms[ti] for ti in tis]
                for ti, (w_sb, xx_sb, pi) in enumerate(tl):
                    for j in range(KP):
                        nc.tensor.matmul(
                            yp[:],
                            lhsT=xx_sb[:, 2 * j : 2 * j + 2, pi, bt],
                            rhs=w_sb[:, 2 * j : 2 * j + 2, fs],
                            start=(start and ti == 0 and j == 0),
                            stop=(stop and ti == len(tl) - 1 and j == KP - 1),
                            perf_mode=DR,
                            skip_group_check=True,
                        )

            def emit_xl_stage2(uxs, yp, half, start, stop):
                fs = slice(512 * half, 512 * (half + 1))
                for pi, ws_sb in ((0, ws1_sb), (1, ws2_sb)):
                    nc.tensor.matmul(
                        yp[:],
                        lhsT=uxs[:, pi, :, :],
                        rhs=ws_sb[:, 0:2, fs],
                        start=(start and pi == 0),
                        stop=(stop and pi == 1),
                        perf_mode=DR,
                        skip_group_check=True,
                    )

            def emit_tail(xs, b, grow, o_sb, yp, half, rs_sb, pieces=1):
                for p in range(pieces):
                    w = 512 // pieces
                    fs = slice(512 * half + p * w, 512 * half + (p + 1) * w)
                    ys = slice(p * w, (p + 1) * w)
                    nc.scalar.activation(
                        o_sb[:, fs], yp[:, ys], AF.Gelu, bias=0.0, scale=rs_sb[:]
                    )
                    if not skip_out_dma:
                        nc.sync.dma_start(
                            out[grow : grow + 128, fs], o_sb[:, fs]
                        )

            # --- chunk bodies. Chunk 0 computes the xl term exactly (its A
            # weights are streaming anyway); chunks 1+ use the truncated-SVD
            # two-stage xl term (stage 1 interleaved into the stats phase).
            for ci in range(NCH):
                if ci + 1 < NCH:
                    make_x_tiles(ci + 1)
                    load_x(ci + 1, ("xh", "xl"))
                xs = {n: x_sb[(n, ci)] for n in ("xh", "xl")}

                rs_t, o_ts, uxs_t = {}, {}, {}
                if ci == 0:
                    vac = {}
                    for b in range(NBT):
                        vac[(b, 0)] = emit_quad(xs, b, 0, f"vu{b}")
                    for b in range(NBT):
                        vac[(b, 1)] = emit_quad(xs, b, 1, f"vm{b}")
                    for b in range(NBT):
                        rs_t[b] = emit_combine(
                            [vac[(b, 0)][0], vac[(b, 1)][0]],
                            [vac[(b, 0)][1], vac[(b, 1)][1]], b,
                        )
                else:
                    for b in range(NBT):
                        va0, muv0 = emit_quad(xs, b, 0, f"vu{b}")
                        va1, muv1 = emit_quad(xs, b, 1, f"vm{b}")
                        uxs_t[b] = emit_xl_stage1(xs, b)
                        rs_t[b] = emit_combine(
                            [va0, va1], [muv0, muv1], b
                        )
                for b in range(NBT):
                    o_ts[b] = outp.tile(
                        [128, D], F32, tag="o", name=f"o_{ci}_{b}"
                    )

                if ci == 0:
                    # head chunk: half-phased so passes trickle behind the
                    # A/xl DMA stream
                    for half in range(2):
                        yts = {}
                        for b in range(NBT):
                            yp = yps.tile([128, 512], F32, tag="y")
                            yts[b] = yp
                            emit_mains_terms(
                                xs, b, yp, half, (0, 1), True, False
                            )
                        for b in range(NBT):
                            emit_mains_terms(
                                xs, b, yts[b], half, (2, 3), False, False
                            )
                        for b in range(NBT):
                            emit_mains_terms(
                                xs, b, yts[b], half, (4, 5), False, True
                            )
                            emit_tail(
                                xs, b, b * 128, o_ts[b], yts[b],
                                half, rs_t[b],
                            )
                else:
                    for b in range(NBT):
                        for half in range(2):
                            yp = yps.tile([128, 512], F32, tag="y")
                            emit_mains_terms(
                                xs, b, yp, half, (0, 1), True, False
                            )
                            emit_xl_stage2(uxs_t[b], yp, half, False, False)
                            emit_mains_terms(
                                xs, b, yp, half, (4, 5), False, True
                            )
                            emit_tail(
                                xs, b, ci * NCHUNK + b * 128, o_ts[b],
                                yp, half, rs_t[b],
                            )

    split_multi_waits(nc)
    return nc


def _q8(x):
    return np.asarray(x, np.float32).astype(nf8)


def fold_weights(inputs):
    f32 = np.float32
    d = D
    wv1 = np.asarray(inputs["w_qkv1"], f32)[2 * d :]
    wv2 = np.asarray(inputs["w_qkv2"], f32)[2 * d :]
    bv1 = np.asarray(inputs["b_qkv1"], f32)[2 * d :]
    bv2 = np.asarray(inputs["b_qkv2"], f32)[2 * d :]
    w_o1 = np.asarray(inputs["w_o1"], f32)
    w_o2 = np.asarray(inputs["w_o2"], f32)
    b_o1 = np.asarray(inputs["b_o1"], f32)
    b_o2 = np.asarray(inputs["b_o2"], f32)
    w_proj = np.asarray(inputs["w_proj"], f32)
    b_proj = np.asarray(inputs["b_proj"], f32)
    g = np.asarray(inputs["ln_g"], f32)
    lb = np.asarray(inputs["ln_b"], f32)

    W1 = w_o1 @ wv1
    W2 = w_o2 @ wv2
    c1 = w_o1 @ bv1 + b_o1
    c2 = w_o2 @ bv2 + b_o2
    Wg = w_proj * g[None, :]
    wbar = Wg.sum(1)
    b2v = w_proj @ lb + b_proj
    A1 = Wg[:, :d] @ W1
    A2 = Wg[:, d:] @ W2
    s1 = W1.T @ np.ones(d, f32)
    s2 = W2.T @ np.ones(d, f32)
    sumc = c1.sum() + c2.sum()
    A1h = A1 - np.outer(wbar, s1) / (2 * d)
    A2h = A2 - np.outer(wbar, s2) / (2 * d)
    chat = Wg[:, :d] @ c1 + Wg[:, d:] @ c2 - (sumc / (2 * d)) * wbar

    tails = []
    Ps = []
    for W in (W1, W2):
        G = W.T @ W
        lam, U = np.linalg.eigh(G)
        lam = lam[::-1]
        U = U[:, ::-1]
        Ps.append(np.sqrt(np.maximum(lam[:R], 0))[:, None] * U[:, :R].T)
        tails.append(lam[R:].sum())
    P1, P2 = Ps

    a1h = _q8(ALPHA * A1h)
    a1l = _q8(ALPHA * A1h - a1h.astype(f32))
    a2h = _q8(ALPHA * A2h)
    a2l = _q8(ALPHA * A2h - a2h.astype(f32))

    # truncated SVD of alpha*Ahat for the chunks-1+ xl term
    vvs, wss = [], []
    for Ah in (A1h, A2h):
        U, S, Vt = np.linalg.svd(ALPHA * Ah)
        sq = np.sqrt(S[:RXL])
        vvs.append(_q8(GU * sq[:, None] * Vt[:RXL]))          # [RXL, D]
        wss.append(_q8((1.0 / GU) * U[:, :RXL] * sq[None, :]))  # [D, RXL]

    svm = np.zeros((d, 4), f32)
    svm[:, 0] = ALPHA * s1 / (2 * d)
    svm[:, 1] = (ALPHA**2 / (2 * d)) * 2 * (W1.T @ c1)
    svm[:, 2] = ALPHA * s2 / (2 * d)
    svm[:, 3] = (ALPHA**2 / (2 * d)) * 2 * (W2.T @ c2)

    consts = {
        "mc": float(ALPHA * sumc / (2 * d)),
        "vc": float(
            (ALPHA**2 / (2 * d)) * (tails[0] + tails[1] + c1 @ c1 + c2 @ c2)
        ),
        "epsp": float(ALPHA**2 * LN_EPS),
        "sqrt_kv": float(np.sqrt(ALPHA**2 / (2 * d)) / GAMMAP),
    }
    has_affine = (np.abs(chat).max() > 0) or (np.abs(b2v).max() > 0)
    assert not has_affine, (
        "nonzero chat/b2 path not emitted; extend build_program for this case"
    )

    # pp pack: per path [P rows | s col | lv col] as rhs columns
    pcols = []
    for pi, P in enumerate((P1, P2)):
        pcols.append(_q8(GAMMAP * P).T)                       # [D, R]
        pcols.append(_q8(svm[:, 2 * pi : 2 * pi + 2]))        # [D, 2]
    shared = {
        "ah1": np.ascontiguousarray(a1h.T),
        "al1": np.ascontiguousarray(a1l.T),
        "ah2": np.ascontiguousarray(a2h.T),
        "al2": np.ascontiguousarray(a2l.T),
        "pp": np.ascontiguousarray(
            np.concatenate(pcols, axis=1).astype(nf8)
        ),
        "vv": np.ascontiguousarray(
            np.concatenate([vvs[0].T, vvs[1].T], axis=1)
        ),
        "ws1": np.ascontiguousarray(wss[0].T),
        "ws2": np.ascontiguousarray(wss[1].T),
    }
    return shared, consts


_CACHED = None


def _get_program(consts=None):
    global _CACHED
    if consts is None:
        assert _CACHED is not None, "program not built yet"
        return _CACHED[1]
    key = tuple(sorted(consts.items()))
    if _CACHED is None or _CACHED[0] != key:
        _CACHED = (key, build_program(consts))
    return _CACHED[1]


def run(inputs, trace=False):
    x_u = np.asarray(inputs["x_u"], np.float32)
    x_m = np.asarray(inputs["x_m"], np.float32)
    shared, consts = fold_weights(inputs)

    xuh = _q8(x_u)
    xul = _q8(x_u - xuh.astype(np.float32))
    xmh = _q8(x_m)
    xml = _q8(x_m - xmh.astype(np.float32))
    # device wants [NCH, D, 2, NCHUNK] packs (path u, m on axis 2)
    xh_pack = np.stack([xuh.T, xmh.T], axis=1)  # [D, 2, B]
    xl_pack = np.stack([xul.T, xml.T], axis=1)

    def core_pack(arr, c):
        sl = arr[:, :, c * BC : (c + 1) * BC]          # [D, 2, BC]
        sl = sl.reshape(D, 2, NCH, NCHUNK)
        return np.ascontiguousarray(sl.transpose(2, 0, 1, 3))

    in_maps = []
    for c in range(N_CORES):
        m = dict(shared)
        m["xh"] = core_pack(xh_pack, c)
        m["xl"] = core_pack(xl_pack, c)
        in_maps.append(m)

    nc = _get_program(consts)
    res = run_bass_kernel_spmd(nc, in_maps, list(range(N_CORES)), trace=trace)
    out = np.empty((B, D), np.float32)
    for c in range(N_CORES):
        out[c * BC : (c + 1) * BC, :] = res.results[c]["out"]
    return out, res


def kernel(**inputs) -> np.ndarray:
    out, _ = run(inputs, trace=False)
    return out
